# revision 1
# baseline (speedup 1.0000x reference)
"""Trainium2 Bass kernel for y = enc_x @ weight.T + bias.

Shapes (hardcoded): enc_x [524288, 128] f32, weight [128, 128] f32,
bias [128] f32 -> y [524288, 128] f32.

Strategy: data-parallel over 8 NeuronCores (65536 rows each). Per core the
kernel streams x through SBUF in [128, 4096] tiles where partition p holds
32 contiguous batch rows (16 KiB contiguous per partition per DMA, which is
the max-bandwidth DMA pattern). The tensor engine contracts over the
partition dim, so each 128x128 block is first PE-transposed (via identity)
into PSUM, copied to SBUF, then used as the stationary operand of a matmul
against W^T (pre-transposed on host). The matmul output lands in natural
[batch, out] layout in PSUM; the bias add is fused into the PSUM->SBUF
eviction (tensor_add against a host-broadcast bias tile). Output DMA uses
the mirror of the input access pattern, so it is also fully contiguous.
"""

import numpy as np

B, IN, OUT = 524288, 128, 128
N_CORES = 8
ROWS = B // N_CORES            # 65536 rows per core
CHUNK = 4096                   # batch rows per SBUF tile
N_CHUNKS = ROWS // CHUNK       # 16
W_PER_P = CHUNK // 128         # 32 rows per partition
FREE = CHUNK                   # SBUF tile free dim (32 blocks of 128)
GROUP = 512                    # PSUM bank: 512 f32 per partition
GROUPS = FREE // GROUP         # 8 groups of 4 blocks

_CACHE: dict = {}


def _build():
    import concourse.bacc as bacc
    import concourse.mybir as mybir
    import concourse.tile as tile
    from concourse.bass import ts

    nc = bacc.Bacc(
        "TRN2",
        target_bir_lowering=False,
        debug=False,
        enable_asserts=False,
        num_devices=N_CORES,
    )

    f32 = mybir.dt.float32
    x_d = nc.dram_tensor("x", [ROWS, IN], f32, kind="ExternalInput").ap()
    wt_d = nc.dram_tensor("wt", [IN, OUT], f32, kind="ExternalInput").ap()
    b4_d = nc.dram_tensor("b4", [128, GROUP], f32, kind="ExternalInput").ap()
    id_d = nc.dram_tensor("ident", [128, 128], f32, kind="ExternalInput").ap()
    y_d = nc.dram_tensor("y", [ROWS, OUT], f32, kind="ExternalOutput").ap()

    # partition p of chunk c holds rows c*4096 + 32p .. 32p+31 (contiguous)
    x_r = x_d.rearrange("(c p w) i -> c p (w i)", p=128, w=W_PER_P)
    y_r = y_d.rearrange("(c p w) o -> c p (w o)", p=128, w=W_PER_P)

    with tile.TileContext(nc) as tc:
        with (
            tc.tile_pool(name="consts", bufs=1) as cpool,
            tc.tile_pool(name="xin", bufs=3) as xpool,
            tc.tile_pool(name="yout", bufs=3) as ypool,
            tc.tile_pool(name="xt", bufs=6) as xtpool,
            tc.tile_pool(name="psT", bufs=3, space="PSUM") as psTpool,
            tc.tile_pool(name="psY", bufs=3, space="PSUM") as psYpool,
        ):
            wt_sb = cpool.tile([128, 128], f32)
            nc.sync.dma_start(wt_sb[:], wt_d)
            b4_sb = cpool.tile([128, GROUP], f32)
            nc.sync.dma_start(b4_sb[:], b4_d)
            id_sb = cpool.tile([128, 128], f32)
            nc.sync.dma_start(id_sb[:], id_d)

            for c in range(N_CHUNKS):
                X = xpool.tile([128, FREE], f32, tag="X")
                nc.sync.dma_start(X[:], x_r[c])
                Y = ypool.tile([128, FREE], f32, tag="Y")
                for g in range(GROUPS):
                    psT = psTpool.tile([128, GROUP], f32, tag="psT")
                    for t in range(4):
                        blk = 4 * g + t
                        nc.tensor.transpose(
                            psT[:, ts(t, 128)], X[:, ts(blk, 128)], id_sb[:]
                        )
                    xT = xtpool.tile([128, GROUP], f32, tag="xT")
                    nc.vector.tensor_copy(xT[:], psT[:])
                    psY = psYpool.tile([128, GROUP], f32, tag="psY")
                    for t in range(4):
                        nc.tensor.matmul(
                            psY[:, ts(t, 128)],
                            xT[:, ts(t, 128)],
                            wt_sb[:],
                            start=True,
                            stop=True,
                        )
                    nc.vector.tensor_add(Y[:, ts(g, GROUP)], psY[:], b4_sb[:])
                nc.sync.dma_start(y_r[c], Y[:])

    nc.compile()
    return nc


def _get_nc():
    if "nc" not in _CACHE:
        _CACHE["nc"] = _build()
    return _CACHE["nc"]


def kernel(enc_x: np.ndarray, weight: np.ndarray, bias: np.ndarray) -> np.ndarray:
    from concourse.bass_utils import run_bass_kernel_spmd

    enc_x = np.ascontiguousarray(enc_x, dtype=np.float32)
    wt = np.ascontiguousarray(weight.T.astype(np.float32))          # [IN, OUT]
    b4 = np.ascontiguousarray(
        np.tile(bias.astype(np.float32)[None, :], (128, GROUP // OUT))
    )                                                               # [128, 512]
    ident = np.eye(128, dtype=np.float32)

    in_maps = [
        {
            "x": enc_x[c * ROWS : (c + 1) * ROWS],
            "wt": wt,
            "b4": b4,
            "ident": ident,
        }
        for c in range(N_CORES)
    ]
    res = run_bass_kernel_spmd(_get_nc(), in_maps, list(range(N_CORES)))
    return np.concatenate([res.results[c]["y"] for c in range(N_CORES)], axis=0)



# revision 2
# speedup vs baseline: 2.0936x; 2.0936x over previous
"""Trainium2 Bass kernel for y = enc_x @ weight.T + bias.

Shapes: enc_x [524288, 128] f32, weight [128, 128] f32, bias [128] f32
-> y [524288, 128] f32.

Strategy: data-parallel over 8 NeuronCores (65536 rows each). The problem is
HBM-bandwidth bound, so all bulk traffic is bf16 (validated: max rel err
~4e-3 vs the 2e-2 gate):

  * host: x slice is cast to bf16 and transposed to xT [128(features), 65536]
    so each partition's DMA span is contiguous (max-bandwidth descriptors) and
    the tensor engine needs NO on-chip transposes.
  * device: stream xT through SBUF in [128, 8192] tiles. For each 512-column
    group, one matmul (stationary = W^T bf16, moving = x^T columns) produces
    yT [out_feature, batch] f32 in PSUM. Eviction PSUM->SBUF fuses the bias
    add (bias is per-partition in this layout) and the bf16 downcast,
    alternating between ScalarE (activation Identity + bias AP) and VectorE
    (tensor_scalar_add) so neither engine is the bottleneck.
  * host: yT bf16 [128, 65536] -> transpose -> f32.

HBM traffic per core: 16 MiB in + 16 MiB out (vs 64 MiB for the f32 kernel).
"""

import numpy as np
import ml_dtypes

B, IN, OUT = 524288, 128, 128
N_CORES = 8
ROWS = B // N_CORES            # 65536 batch rows per core
CHUNK_F = 8192                 # batch columns per SBUF tile
N_CHUNKS = ROWS // CHUNK_F     # 8
GROUP = 512                    # PSUM bank: 512 f32 per partition
GROUPS = CHUNK_F // GROUP      # 16 matmul groups per chunk

BF16 = ml_dtypes.bfloat16

_CACHE: dict = {}


def _build():
    import concourse.bacc as bacc
    import concourse.mybir as mybir
    import concourse.tile as tile
    from concourse.bass import ts

    nc = bacc.Bacc(
        "TRN2",
        target_bir_lowering=False,
        debug=False,
        enable_asserts=False,
        num_devices=N_CORES,
    )

    f32 = mybir.dt.float32
    bf16 = mybir.dt.bfloat16
    xT_d = nc.dram_tensor("xT", [IN, ROWS], bf16, kind="ExternalInput").ap()
    wt_d = nc.dram_tensor("wt", [IN, OUT], bf16, kind="ExternalInput").ap()
    b_d = nc.dram_tensor("bias", [OUT, 1], f32, kind="ExternalInput").ap()
    yT_d = nc.dram_tensor("yT", [OUT, ROWS], bf16, kind="ExternalOutput").ap()

    x_r = xT_d.rearrange("p (c f) -> c p f", f=CHUNK_F)
    y_r = yT_d.rearrange("p (c f) -> c p f", f=CHUNK_F)

    ident = mybir.ActivationFunctionType.Identity

    with tile.TileContext(nc) as tc:
        with (
            tc.tile_pool(name="consts", bufs=1) as cpool,
            tc.tile_pool(name="xin", bufs=3) as xpool,
            tc.tile_pool(name="yout", bufs=3) as ypool,
            tc.tile_pool(name="ps", bufs=6, space="PSUM") as pspool,
        ):
            wt_sb = cpool.tile([IN, OUT], bf16)
            nc.sync.dma_start(wt_sb[:], wt_d)
            b_sb = cpool.tile([OUT, 1], f32)
            nc.sync.dma_start(b_sb[:], b_d)

            for c in range(N_CHUNKS):
                X = xpool.tile([128, CHUNK_F], bf16, tag="X")
                nc.sync.dma_start(X[:], x_r[c])
                Y = ypool.tile([128, CHUNK_F], bf16, tag="Y")
                for g in range(GROUPS):
                    ps = pspool.tile([128, GROUP], f32, tag="ps")
                    nc.tensor.matmul(
                        ps[:], wt_sb[:], X[:, ts(g, GROUP)], start=True, stop=True
                    )
                    if g % 2 == 0:
                        nc.scalar.activation(
                            Y[:, ts(g, GROUP)], ps[:], ident, bias=b_sb[:]
                        )
                    else:
                        nc.vector.tensor_scalar_add(Y[:, ts(g, GROUP)], ps[:], b_sb[:])
                nc.sync.dma_start(y_r[c], Y[:])

    nc.compile()
    return nc


def _get_nc():
    if "nc" not in _CACHE:
        _CACHE["nc"] = _build()
    return _CACHE["nc"]


def _cast_T(x: np.ndarray) -> np.ndarray:
    """[R, 128] f32 -> [128, R] bf16, blocked for cache locality."""
    out = np.empty((IN, x.shape[0]), dtype=BF16)
    step = 4096
    for i in range(0, x.shape[0], step):
        out[:, i : i + step] = x[i : i + step].astype(BF16).T
    return out


def make_in_maps(enc_x: np.ndarray, weight: np.ndarray, bias: np.ndarray):
    wt = np.ascontiguousarray(weight.astype(np.float32).T.astype(BF16))  # [IN, OUT]
    b_col = np.ascontiguousarray(bias.astype(np.float32).reshape(OUT, 1))
    return [
        {
            "xT": _cast_T(enc_x[c * ROWS : (c + 1) * ROWS]),
            "wt": wt,
            "bias": b_col,
        }
        for c in range(N_CORES)
    ]


def kernel(enc_x: np.ndarray, weight: np.ndarray, bias: np.ndarray) -> np.ndarray:
    from concourse.bass_utils import run_bass_kernel_spmd

    enc_x = np.ascontiguousarray(enc_x, dtype=np.float32)
    in_maps = make_in_maps(enc_x, weight, bias)
    res = run_bass_kernel_spmd(_get_nc(), in_maps, list(range(N_CORES)))
    y = np.empty((B, OUT), dtype=np.float32)
    for c in range(N_CORES):
        y[c * ROWS : (c + 1) * ROWS] = res.results[c]["yT"].T.astype(np.float32)
    return y


# revision 4
# speedup vs baseline: 2.4844x; 1.1867x over previous
"""Trainium2 Bass kernel for y = enc_x @ weight.T + bias.

Shapes: enc_x [524288, 128] f32, weight [128, 128] f32, bias [128] f32
-> y [524288, 128] f32.

Strategy: data-parallel over 8 NeuronCores (65536 rows each). The problem is
HBM-bandwidth bound, so all bulk traffic is bf16 (validated: max rel err
~4e-3 vs the 2e-2 gate):

  * host: x slice is cast to bf16 and transposed to xT [128(features), 65536]
    so each partition's DMA span is contiguous (max-bandwidth descriptors) and
    the tensor engine needs NO on-chip transposes.
  * device: stream xT through SBUF in [128, 8192] tiles. For each 512-column
    group, one matmul (stationary = W^T bf16, moving = x^T columns) produces
    yT [out_feature, batch] f32 in PSUM. Eviction PSUM->SBUF fuses the bias
    add (bias is per-partition in this layout) and the bf16 downcast,
    alternating between ScalarE (activation Identity + bias AP) and VectorE
    (tensor_scalar_add) so neither engine is the bottleneck.
  * host: yT bf16 [128, 65536] -> transpose -> f32.

HBM traffic per core: 16 MiB in + 16 MiB out (vs 64 MiB for the f32 kernel).
"""

import numpy as np
import ml_dtypes

B, IN, OUT = 524288, 128, 128
N_CORES = 8
ROWS = B // N_CORES            # 65536 batch rows per core
CHUNK_F = 4096                 # batch columns per SBUF tile
N_CHUNKS = ROWS // CHUNK_F     # 16
GROUP = 512                    # PSUM bank: 512 f32 per partition
EVICT = 1024                   # eviction granularity (2 PSUM banks)
EVICTS = CHUNK_F // EVICT      # 4 evictions per chunk

BF16 = ml_dtypes.bfloat16

_CACHE: dict = {}


def _build():
    import concourse.bacc as bacc
    import concourse.mybir as mybir
    import concourse.tile as tile
    from concourse.bass import ts

    nc = bacc.Bacc(
        "TRN2",
        target_bir_lowering=False,
        debug=False,
        enable_asserts=False,
        num_devices=N_CORES,
    )

    f32 = mybir.dt.float32
    bf16 = mybir.dt.bfloat16
    xT_d = nc.dram_tensor("xT", [IN, ROWS], bf16, kind="ExternalInput").ap()
    wt_d = nc.dram_tensor("wt", [IN, OUT], bf16, kind="ExternalInput").ap()
    b_d = nc.dram_tensor("bias", [OUT, 1], f32, kind="ExternalInput").ap()
    yT_d = nc.dram_tensor("yT", [OUT, ROWS], bf16, kind="ExternalOutput").ap()

    x_r = xT_d.rearrange("p (c f) -> c p f", f=CHUNK_F)
    y_r = yT_d.rearrange("p (c f) -> c p f", f=CHUNK_F)

    ident = mybir.ActivationFunctionType.Identity

    with tile.TileContext(nc) as tc:
        with (
            tc.tile_pool(name="consts", bufs=1) as cpool,
            tc.tile_pool(name="xin", bufs=6) as xpool,
            tc.tile_pool(name="yout", bufs=6) as ypool,
            tc.tile_pool(name="ps", bufs=3, space="PSUM") as pspool,
        ):
            # First x tile before the (tiny) consts so its descriptors lead
            # the HWDGE ring; wt/bias packets interleave and still arrive
            # within the first chunk's stream time.
            X0 = xpool.tile([128, CHUNK_F], bf16, tag="X")
            nc.sync.dma_start(X0[:], x_r[0])
            wt_sb = cpool.tile([IN, OUT], bf16)
            nc.sync.dma_start(wt_sb[:], wt_d)
            b_sb = cpool.tile([OUT, 1], f32)
            nc.sync.dma_start(b_sb[:], b_d)

            for c in range(N_CHUNKS):
                if c == 0:
                    X = X0
                else:
                    X = xpool.tile([128, CHUNK_F], bf16, tag="X")
                    nc.sync.dma_start(X[:], x_r[c])
                Y = ypool.tile([128, CHUNK_F], bf16, tag="Y")
                for e in range(EVICTS):
                    ps = pspool.tile([128, EVICT], f32, tag="ps")
                    for h in range(EVICT // GROUP):
                        g = e * (EVICT // GROUP) + h
                        nc.tensor.matmul(
                            ps[:, ts(h, GROUP)],
                            wt_sb[:],
                            X[:, ts(g, GROUP)],
                            start=True,
                            stop=True,
                        )
                    if e % 2 == 0:
                        nc.scalar.activation(
                            Y[:, ts(e, EVICT)], ps[:], ident, bias=b_sb[:]
                        )
                    else:
                        nc.vector.tensor_scalar_add(Y[:, ts(e, EVICT)], ps[:], b_sb[:])
                # out-DMA rides the GpSimd (SWDGE) ring so its semaphore wait
                # never head-of-line-blocks the Sync ring feeding x tiles.
                nc.gpsimd.dma_start(y_r[c], Y[:])

    nc.compile()
    return nc


def _get_nc():
    if "nc" not in _CACHE:
        _CACHE["nc"] = _build()
    return _CACHE["nc"]


def _cast_T(x: np.ndarray) -> np.ndarray:
    """[R, 128] f32 -> [128, R] bf16, blocked for cache locality."""
    out = np.empty((IN, x.shape[0]), dtype=BF16)
    step = 4096
    for i in range(0, x.shape[0], step):
        out[:, i : i + step] = x[i : i + step].astype(BF16).T
    return out


def make_in_maps(enc_x: np.ndarray, weight: np.ndarray, bias: np.ndarray):
    wt = np.ascontiguousarray(weight.astype(np.float32).T.astype(BF16))  # [IN, OUT]
    b_col = np.ascontiguousarray(bias.astype(np.float32).reshape(OUT, 1))
    return [
        {
            "xT": _cast_T(enc_x[c * ROWS : (c + 1) * ROWS]),
            "wt": wt,
            "bias": b_col,
        }
        for c in range(N_CORES)
    ]


def kernel(enc_x: np.ndarray, weight: np.ndarray, bias: np.ndarray) -> np.ndarray:
    from concourse.bass_utils import run_bass_kernel_spmd

    enc_x = np.ascontiguousarray(enc_x, dtype=np.float32)
    in_maps = make_in_maps(enc_x, weight, bias)
    res = run_bass_kernel_spmd(_get_nc(), in_maps, list(range(N_CORES)))
    y = np.empty((B, OUT), dtype=np.float32)
    for c in range(N_CORES):
        y[c * ROWS : (c + 1) * ROWS] = res.results[c]["yT"].T.astype(np.float32)
    return y
